# revision 2
# baseline (speedup 1.0000x reference)
"""Banded local-linear layer (nn_LocalLinearLayer) on 8 trn2 NeuronCores.

out[b, o, c] = sum_p W[o, p] * xpad[b, c, p] + bias[o],  band p in [o, o+25)
xpad = edge-replicate pad of x along L (first/last 12 rows duplicated).

Strategy (v3):
  - Data-parallel over batch: 4 batches per core; banded weights replicated.
  - Output tiled in 104-row tiles: tile t = out rows [104t, 104t+104), contracts
    over xpad rows [104t, 104t+128) -> ONE K=128 matmul per tile (40 tiles).
  - Host pre-shuffles xpad into the exact SBUF layout [128, tile, b*64+c] (fp16)
    and unshuffles the output, so every DMA is fully contiguous (large
    descriptors, no strided-DMA penalty) and the device loop is uniform.
  - fp16 operands, fp32 PSUM accumulation, fp32 bias/output (~4e-4 rel err).
  - PSUM->SBUF + bias alternates ScalarE activation / VectorE tensor_scalar_add.
  - x/out staged in 4 chunks of 10 tiles for DMA/compute overlap; input DMAs on
    the Sync HWDGE ring, output DMAs on the Scalar ring.
"""

import sys

for _p in ("/opt/trn_rl_repo",):
    if _p not in sys.path:
        sys.path.insert(0, _p)

import numpy as np

import concourse.bass as bass
import concourse.tile as tile
from concourse import bacc, mybir
from concourse.bass_utils import run_bass_kernel_spmd

L = 4096
WIN = 25
PAD = (WIN - 1) // 2  # 12
PADDED = L + 2 * PAD  # 4120
B = 32
C = 64
NCORES = 8
BPC = B // NCORES  # 4
P = 128
M = P - (WIN - 1)  # 104 output rows per tile
NT = (L + M - 1) // M  # 40 tiles
M_LAST = L - (NT - 1) * M  # 40
K_LAST = PADDED - (NT - 1) * M  # 64
NFREE = BPC * C  # 256
NCHUNK = 4
TPC = NT // NCHUNK  # 10

F32 = mybir.dt.float32
F16 = mybir.dt.float16


def _host_weights(W: np.ndarray, b: np.ndarray):
    o = np.arange(L)[:, None]
    p = np.arange(PADDED)[None, :]
    Wm = np.where((p >= o) & (p < o + WIN), W, 0.0).astype(np.float32)
    # wb[k, t, m] = Wm[t*104+m, t*104+k], zero-padded out of range
    wb = np.zeros((P, NT, M), np.float32)
    bias_t = np.zeros((M, NT), np.float32)
    for t in range(NT):
        mt = min(M, L - t * M)
        kt = min(P, PADDED - t * M)
        wb[:kt, t, :mt] = Wm[t * M : t * M + mt, t * M : t * M + kt].T
        bias_t[:mt, t] = b[t * M : t * M + mt]
    return wb.astype(np.float16), bias_t


def _host_x(x: np.ndarray):
    """x [B, L, C] f32 -> [P, NT, B, C] f16 in xpad-tile layout."""
    xp = np.concatenate([x[:, :PAD], x, x[:, -PAD:]], axis=1).astype(np.float16)
    xh = np.zeros((P, NT, B, C), np.float16)
    for t in range(NT):
        kt = min(P, PADDED - t * M)
        xh[:kt, t] = xp[:, t * M : t * M + kt].transpose(1, 0, 2)
    return xh


def _build_nc():
    nc = bacc.Bacc("TRN2", target_bir_lowering=False, debug=False, num_devices=NCORES)
    x_d = nc.dram_tensor("x", [P, NT, NFREE], F16, kind="ExternalInput").ap()
    wb_d = nc.dram_tensor("wb", [P, NT, M], F16, kind="ExternalInput").ap()
    bias_d = nc.dram_tensor("bias", [M, NT], F32, kind="ExternalInput").ap()
    out_d = nc.dram_tensor("out", [M, NT, NFREE], F32, kind="ExternalOutput").ap()

    with tile.TileContext(nc) as tc:
        with (
            tc.tile_pool(name="main", bufs=1) as pool,
            tc.tile_pool(name="ps", bufs=8, space=bass.MemorySpace.PSUM) as pspool,
        ):
            wb_s = pool.tile([P, NT, M], F16)
            bias_s = pool.tile([M, NT], F32)
            xch = [
                pool.tile([P, TPC, NFREE], F16, name=f"xch{c}") for c in range(NCHUNK)
            ]
            sch = [
                pool.tile([M, TPC, NFREE], F32, name=f"sch{c}") for c in range(NCHUNK)
            ]

            nc.sync.dma_start(wb_s[:], wb_d)
            nc.sync.dma_start(bias_s[:], bias_d)
            for ch in range(NCHUNK):
                nc.sync.dma_start(
                    xch[ch][:], x_d[:, ch * TPC : (ch + 1) * TPC, :]
                )

            for t in range(NT):
                c, j = t // TPC, t % TPC
                ps = pspool.tile([M, NFREE], F32)
                nc.tensor.matmul(
                    ps[:], wb_s[:, t], xch[c][:, j, :], start=True, stop=True
                )
                if t % 2 == 0:
                    nc.scalar.activation(
                        sch[c][:, j, :],
                        ps[:],
                        mybir.ActivationFunctionType.Identity,
                        bias=bias_s[:, t : t + 1],
                    )
                else:
                    nc.vector.tensor_scalar_add(
                        sch[c][:, j, :], ps[:], bias_s[:, t : t + 1]
                    )

            for ch in range(NCHUNK):
                nc.scalar.dma_start(
                    out_d[:, ch * TPC : (ch + 1) * TPC, :], sch[ch][:]
                )

    nc.compile()
    return nc


_NC = None


def _get_nc():
    global _NC
    if _NC is None:
        _NC = _build_nc()
    return _NC


def _make_in_maps(x, W, b):
    wb, bias_t = _host_weights(
        np.asarray(W, dtype=np.float32), np.asarray(b, dtype=np.float32)
    )
    xh = _host_x(np.asarray(x, dtype=np.float32))
    return [
        {
            "x": np.ascontiguousarray(
                xh[:, :, c * BPC : (c + 1) * BPC, :]
            ).reshape(P, NT, NFREE),
            "wb": wb,
            "bias": bias_t,
        }
        for c in range(NCORES)
    ]


def _gather(results):
    oh = np.concatenate(
        [r["out"].reshape(M, NT, BPC, C) for r in results], axis=2
    )  # [104, NT, B, C]
    out = np.empty((B, L, C), np.float32)
    for t in range(NT):
        mt = min(M, L - t * M)
        out[:, t * M : t * M + mt] = oh[:mt, t].transpose(1, 0, 2)
    return out


def kernel(x: np.ndarray, W: np.ndarray, b: np.ndarray) -> np.ndarray:
    nc = _get_nc()
    res = run_bass_kernel_spmd(nc, _make_in_maps(x, W, b), list(range(NCORES)))
    return _gather(res.results)


if __name__ == "__main__":
    rng = np.random.default_rng(0)
    x = rng.standard_normal((B, L, C), dtype=np.float32)
    W = rng.standard_normal((L, PADDED), dtype=np.float32) * 0.02
    b = rng.standard_normal((L,), dtype=np.float32) * 0.02
    print(kernel(x, W, b).shape)



# revision 8
# speedup vs baseline: 1.2943x; 1.2943x over previous
"""Banded local-linear layer (nn_LocalLinearLayer) on 8 trn2 NeuronCores.

out[b, o, c] = sum_p W[o, p] * xpad[b, c, p] + bias[o],  band p in [o, o+25)
xpad = edge-replicate pad of x along L (first/last 12 rows duplicated).

Strategy (v5):
  - Tensor-parallel over output rows: core c owns out rows [512c, 512c+512)
    for ALL batches/channels -> banded weight is sharded 8-way (134 KB/core)
    instead of replicated (1.06 MB/core).
  - x stored in SBUF as non-overlapping 128-row tiles of xpad (5 tiles/core,
    the 5th is the 24-row halo into the next core's range).  Each 128-row
    output tile T is ONE K=128 banded matmul vs x-tile T plus a tiny K=24
    triangular "corner" matmul vs the first 24 rows of x-tile T+1,
    accumulated in PSUM.  No duplicated x traffic, and output tiles use all
    128 partitions.
  - bf16 operands (fp16 matmul streams at half rate on trn2), fp32 PSUM,
    fp16 output (engines cast during the PSUM->SBUF bias-add copy).
  - Per-tile output DMAs issued on GpSimd (SWDGE) interleaved with compute
    so stores start as soon as tile 0 is done; x loads ride the Sync HWDGE
    ring, small weight/bias loads the Scalar HWDGE ring.
  - 8 dummy matmuls on a zeroed scratch tile run during the input-DMA wait
    to lift the PE HAM clock gate (1.2 -> 2.4 GHz) before real work.
"""

import sys

for _p in ("/opt/trn_rl_repo",):
    if _p not in sys.path:
        sys.path.insert(0, _p)

import ml_dtypes
import numpy as np

import concourse.bass as bass
import concourse.tile as tile
from concourse import bacc, mybir
from concourse.bass_utils import run_bass_kernel_spmd

L = 4096
WIN = 25
PAD = (WIN - 1) // 2  # 12
PADDED = L + 2 * PAD  # 4120
B = 32
C = 64
NCORES = 8
P = 128
ROWS_PC = L // NCORES  # 512 output rows per core
NT = ROWS_PC // P  # 4 output tiles per core
NXT = NT + 1  # 5 x tiles per core (incl. 24-row halo tile)
NFREE = B * C  # 2048
CK = WIN - 1  # 24 = corner contraction size
CM = 32  # corner output columns (PSUM base must be 32-aligned -> rows [96,128))
NB = NFREE // 512  # 4 psum banks per tile

F32 = mybir.dt.float32
F16 = mybir.dt.float16
BF16 = mybir.dt.bfloat16
NPBF16 = np.dtype(ml_dtypes.bfloat16)


def _host_weights(W: np.ndarray, b: np.ndarray):
    """Band-extract and shard W/b by output row.

    Returns per-core lists: w1 [128, NT, 128] bf16, w2 [CK, NT, CK] bf16,
    bias [128, NT] f32.
    wk[j, o] = W[o, o+j] is the dense band (j in [0, WIN)).
    w1[k, T, m] = wk[k-m, o0+m]        for 0 <= k-m < WIN  (o0 = core/tile base)
    w2[k', T, mp] = wk[32+k'-mp, o0+96+mp] for k'+8 <= mp < 32 (rows of tile T+1;
    out rows [96,128), first 8 columns zero so the PSUM base stays 32-aligned)
    """
    o = np.arange(L)
    wk = W[o[:, None], o[:, None] + np.arange(WIN)[None, :]].T  # [WIN, L]
    w1s, w2s, biass = [], [], []
    for c in range(NCORES):
        w1 = np.zeros((P, NT, P), np.float32)
        w2 = np.zeros((CK, NT, CM), np.float32)
        bias = np.zeros((P, NT), np.float32)
        for T in range(NT):
            o0 = c * ROWS_PC + T * P
            for j in range(WIN):
                m = np.arange(0, P - j)
                w1[m + j, T, m] = wk[j, o0 + m]
            for mp in range(8, CM):
                kp = np.arange(0, mp - 8 + 1)
                w2[kp, T, mp] = wk[CM + kp - mp, o0 + (P - CM) + mp]
            bias[:, T] = b[o0 : o0 + P]
        w1s.append(w1.astype(NPBF16))
        w2s.append(w2.astype(NPBF16))
        biass.append(bias)
    return w1s, w2s, biass


def _host_x(x: np.ndarray):
    """x [B, L, C] f32 -> per-core [128, NXT, B*C] bf16 tiles of xpad."""
    xp = np.concatenate([x[:, :PAD], x, x[:, -PAD:]], axis=1)  # [B, PADDED, C]
    xpad = np.zeros((B, NCORES * ROWS_PC + P, C), np.float32)  # 4224 rows
    xpad[:, :PADDED] = xp
    xh = []
    for c in range(NCORES):
        sl = xpad[:, c * ROWS_PC : c * ROWS_PC + NXT * P]  # [B, 640, C]
        t = sl.reshape(B, NXT, P, C).transpose(2, 1, 0, 3).reshape(P, NXT, NFREE)
        xh.append(np.ascontiguousarray(t.astype(NPBF16)))
    return xh


def _build_nc():
    nc = bacc.Bacc("TRN2", target_bir_lowering=False, debug=False, num_devices=NCORES)
    x_d = nc.dram_tensor("x", [P, NXT, NFREE], BF16, kind="ExternalInput").ap()
    w1_d = nc.dram_tensor("w1", [P, NT, P], BF16, kind="ExternalInput").ap()
    w2_d = nc.dram_tensor("w2", [CK, NT, CM], BF16, kind="ExternalInput").ap()
    bias_d = nc.dram_tensor("bias", [P, NT], F32, kind="ExternalInput").ap()
    out_d = nc.dram_tensor("out", [P, NT, NFREE], F16, kind="ExternalOutput").ap()

    with tile.TileContext(nc) as tc:
        with (
            tc.tile_pool(name="main", bufs=1) as pool,
            tc.tile_pool(name="ps", bufs=2, space=bass.MemorySpace.PSUM) as pspool,
        ):
            x_s = pool.tile([P, NXT, NFREE], BF16)
            w1_s = pool.tile([P, NT, P], BF16)
            w2_s = pool.tile([CK, NT, CM], BF16)
            bias_s = pool.tile([P, NT], F32)
            out_s = pool.tile([P, NT, NFREE], F16)
            scratch = pool.tile([P, 512], BF16)

            # big x loads on the Sync HWDGE ring, small loads on Scalar's
            nc.sync.dma_start(x_s[:, 0:2], x_d[:, 0:2])
            nc.sync.dma_start(x_s[:, 2:4], x_d[:, 2:4])
            nc.sync.dma_start(x_s[:, 4:5], x_d[:, 4:5])
            nc.scalar.dma_start(w1_s[:], w1_d)
            nc.scalar.dma_start(w2_s[:], w2_d)
            nc.scalar.dma_start(bias_s[:], bias_d)

            # PE clock warm-up: dummy matmuls on zeroed scratch, no DMA deps
            nc.vector.memset(scratch[:], 0.0)
            ps_warm = pspool.tile([P, NFREE], F32, tag="psacc")
            for _ in range(8):
                nc.tensor.matmul(
                    ps_warm[:, 0:512], scratch[:, 0:P], scratch[:], start=True, stop=True
                )

            for T in range(NT):
                ps = pspool.tile([P, NFREE], F32, tag="psacc")
                for F in range(NB):
                    s = slice(F * 512, (F + 1) * 512)
                    nc.tensor.matmul(
                        ps[:, s], w1_s[:, T], x_s[:, T, s], start=True, stop=True
                    )
                for F in range(NB):
                    s = slice(F * 512, (F + 1) * 512)
                    nc.tensor.matmul(
                        ps[P - CM : P, s],
                        w2_s[:, T],
                        x_s[0:CK, T + 1, s],
                        start=False,
                        stop=True,
                        tile_position=(0, P - CM),
                        skip_group_check=True,
                    )
                half = NFREE // 2
                nc.scalar.activation(
                    out_s[:, T, 0:half],
                    ps[:, 0:half],
                    mybir.ActivationFunctionType.Identity,
                    bias=bias_s[:, T : T + 1],
                )
                nc.vector.tensor_scalar_add(
                    out_s[:, T, half:NFREE], ps[:, half:NFREE], bias_s[:, T : T + 1]
                )
                nc.gpsimd.dma_start(out_d[:, T], out_s[:, T])

    nc.compile()
    return nc


_NC = None


def _get_nc():
    global _NC
    if _NC is None:
        _NC = _build_nc()
    return _NC


def _make_in_maps(x, W, b):
    w1s, w2s, biass = _host_weights(
        np.asarray(W, dtype=np.float32), np.asarray(b, dtype=np.float32)
    )
    xh = _host_x(np.asarray(x, dtype=np.float32))
    return [
        {"x": xh[c], "w1": w1s[c], "w2": w2s[c], "bias": biass[c]}
        for c in range(NCORES)
    ]


def _gather(results):
    out = np.empty((B, L, C), np.float32)
    for c, r in enumerate(results):
        # r["out"]: [128, NT, B*C] f16 -> [B, 512, C]
        t = np.asarray(r["out"]).reshape(P, NT, B, C).transpose(2, 1, 0, 3)
        out[:, c * ROWS_PC : (c + 1) * ROWS_PC] = t.reshape(B, ROWS_PC, C)
    return out


def kernel(x: np.ndarray, W: np.ndarray, b: np.ndarray) -> np.ndarray:
    nc = _get_nc()
    res = run_bass_kernel_spmd(nc, _make_in_maps(x, W, b), list(range(NCORES)))
    return _gather(res.results)


if __name__ == "__main__":
    rng = np.random.default_rng(0)
    x = rng.standard_normal((B, L, C), dtype=np.float32)
    W = rng.standard_normal((L, PADDED), dtype=np.float32) * 0.02
    b = rng.standard_normal((L,), dtype=np.float32) * 0.02
    print(kernel(x, W, b).shape)


# revision 9
# speedup vs baseline: 1.4708x; 1.1363x over previous
"""Banded local-linear layer (nn_LocalLinearLayer) on 8 trn2 NeuronCores.

out[b, o, c] = sum_p W[o, p] * xpad[b, c, p] + bias[o],  band p in [o, o+25)
xpad = edge-replicate pad of x along L (first/last 12 rows duplicated).

Strategy (v6):
  - Tensor-parallel over output rows: core c owns out rows [512c, 512c+512)
    for ALL batches/channels -> banded weight is sharded 8-way (~136 KB/core)
    instead of replicated (1.06 MB/core).
  - 5 output tiles per core of M=104 rows (last 96): tile t contracts xpad
    rows [104t, 104t+128) -> ONE K=128 banded weight per tile, streamed as 4
    PSUM-bank matmuls of 512 columns.  No tiny corner matmuls (they stream
    full columns for 32 rows and read as idle to the PE clock governor).
  - bf16 operands (1 col/cycle on the PE), fp32 PSUM, fp16 output.
  - x tiles DMA'd individually on the Sync HWDGE ring so tile t's matmuls
    start as soon as its rows land; weights/bias ride the Scalar HWDGE ring;
    per-tile output DMAs ride the GpSimd SWDGE ring (measured 362 GB/s).
  - PSUM->SBUF bias-add copies split ScalarE/VectorE half-and-half.
  - 4 dummy matmuls on a zeroed scratch tile lift the PE HAM clock gate
    (1.2 -> 2.4 GHz) during the initial DMA wait.
"""

import sys

for _p in ("/opt/trn_rl_repo",):
    if _p not in sys.path:
        sys.path.insert(0, _p)

import ml_dtypes
import numpy as np

import concourse.bass as bass
import concourse.tile as tile
from concourse import bacc, mybir
from concourse.bass_utils import run_bass_kernel_spmd

L = 4096
WIN = 25
PAD = (WIN - 1) // 2  # 12
PADDED = L + 2 * PAD  # 4120
B = 32
C = 64
NCORES = 8
P = 128
ROWS_PC = L // NCORES  # 512 output rows per core
M = P - (WIN - 1)  # 104 output rows per tile
NT = (ROWS_PC + M - 1) // M  # 5 tiles per core
M_LAST = ROWS_PC - (NT - 1) * M  # 96
NFREE = B * C  # 2048
NB = NFREE // 512  # 4 psum banks per tile

F32 = mybir.dt.float32
F16 = mybir.dt.float16
BF16 = mybir.dt.bfloat16
NPBF16 = np.dtype(ml_dtypes.bfloat16)


def _host_weights(W: np.ndarray, b: np.ndarray):
    """Band-extract and shard W/b by output row.

    wk[j, o] = W[o, o+j] is the dense band (j in [0, WIN)).
    Per core: w1[k, t, m] = wk[k-m, o0+m] for 0 <= k-m < WIN (o0 = 512c+104t),
    bias[m, t] = b[o0+m].  m beyond the tile's rows stays zero.
    """
    o = np.arange(L)
    wk = W[o[:, None], o[:, None] + np.arange(WIN)[None, :]].T  # [WIN, L]
    w1s, biass = [], []
    for c in range(NCORES):
        w1 = np.zeros((P, NT, M), np.float32)
        bias = np.zeros((M, NT), np.float32)
        for t in range(NT):
            o0 = c * ROWS_PC + t * M
            mt = min(M, ROWS_PC - t * M)
            for j in range(WIN):
                m = np.arange(0, mt)
                w1[m + j, t, m] = wk[j, o0 + m]
            bias[:mt, t] = b[o0 : o0 + mt]
        w1s.append(w1.astype(NPBF16))
        biass.append(bias)
    return w1s, biass


def _host_x(x: np.ndarray):
    """x [B, L, C] f32 -> per-core [128, NT, B*C] bf16 overlapping xpad tiles."""
    xp = np.concatenate([x[:, :PAD], x, x[:, -PAD:]], axis=1)  # [B, PADDED, C]
    xpad = np.zeros((B, NCORES * ROWS_PC + P, C), np.float32)
    xpad[:, :PADDED] = xp
    xh = []
    for c in range(NCORES):
        t = np.empty((P, NT, B, C), np.float32)
        for ti in range(NT):
            r0 = c * ROWS_PC + ti * M
            t[:, ti] = xpad[:, r0 : r0 + P].transpose(1, 0, 2)
        xh.append(
            np.ascontiguousarray(t.reshape(P, NT, NFREE).astype(NPBF16))
        )
    return xh


def _build_nc():
    nc = bacc.Bacc("TRN2", target_bir_lowering=False, debug=False, num_devices=NCORES)
    x_d = nc.dram_tensor("x", [P, NT, NFREE], BF16, kind="ExternalInput").ap()
    w1_d = nc.dram_tensor("w1", [P, NT, M], BF16, kind="ExternalInput").ap()
    bias_d = nc.dram_tensor("bias", [M, NT], F32, kind="ExternalInput").ap()
    out_d = nc.dram_tensor("out", [M, NT, NFREE], F16, kind="ExternalOutput").ap()

    with tile.TileContext(nc) as tc:
        with (
            tc.tile_pool(name="main", bufs=1) as pool,
            tc.tile_pool(name="ps", bufs=2, space=bass.MemorySpace.PSUM) as pspool,
        ):
            x_s = pool.tile([P, NT, NFREE], BF16)
            w1_s = pool.tile([P, NT, M], BF16)
            bias_s = pool.tile([M, NT], F32)
            out_s = pool.tile([M, NT, NFREE], F16)
            scratch = pool.tile([P, 512], BF16)

            # per-tile x loads on the Sync HWDGE ring; small loads on Scalar's
            for t in range(NT):
                nc.sync.dma_start(x_s[:, t : t + 1], x_d[:, t : t + 1])
            nc.scalar.dma_start(w1_s[:], w1_d)
            nc.scalar.dma_start(bias_s[:], bias_d)

            # PE clock warm-up: dummy matmuls on zeroed scratch, no DMA deps
            nc.vector.memset(scratch[:], 0.0)
            ps_warm = pspool.tile([P, NFREE], F32, tag="psacc")
            for _ in range(4):
                nc.tensor.matmul(
                    ps_warm[:, 0:512], scratch[:, 0:P], scratch[:], start=True, stop=True
                )

            half = NFREE // 2
            for t in range(NT):
                ps = pspool.tile([P, NFREE], F32, tag="psacc")
                for F in range(NB):
                    s = slice(F * 512, (F + 1) * 512)
                    nc.tensor.matmul(
                        ps[0:M, s], w1_s[:, t], x_s[:, t, s], start=True, stop=True
                    )
                nc.scalar.activation(
                    out_s[:, t, 0:half],
                    ps[0:M, 0:half],
                    mybir.ActivationFunctionType.Identity,
                    bias=bias_s[:, t : t + 1],
                )
                nc.vector.tensor_scalar_add(
                    out_s[:, t, half:NFREE], ps[0:M, half:NFREE], bias_s[:, t : t + 1]
                )
                nc.gpsimd.dma_start(out_d[:, t], out_s[:, t])

    nc.compile()
    return nc


_NC = None


def _get_nc():
    global _NC
    if _NC is None:
        _NC = _build_nc()
    return _NC


def _make_in_maps(x, W, b):
    w1s, biass = _host_weights(
        np.asarray(W, dtype=np.float32), np.asarray(b, dtype=np.float32)
    )
    xh = _host_x(np.asarray(x, dtype=np.float32))
    return [
        {"x": xh[c], "w1": w1s[c], "bias": biass[c]} for c in range(NCORES)
    ]


def _gather(results):
    out = np.empty((B, L, C), np.float32)
    for c, r in enumerate(results):
        oh = np.asarray(r["out"]).reshape(M, NT, B, C)  # [104, 5, B, C]
        for t in range(NT):
            mt = min(M, ROWS_PC - t * M)
            r0 = c * ROWS_PC + t * M
            out[:, r0 : r0 + mt] = oh[:mt, t].transpose(1, 0, 2)
    return out


def kernel(x: np.ndarray, W: np.ndarray, b: np.ndarray) -> np.ndarray:
    nc = _get_nc()
    res = run_bass_kernel_spmd(nc, _make_in_maps(x, W, b), list(range(NCORES)))
    return _gather(res.results)


if __name__ == "__main__":
    rng = np.random.default_rng(0)
    x = rng.standard_normal((B, L, C), dtype=np.float32)
    W = rng.standard_normal((L, PADDED), dtype=np.float32) * 0.02
    b = rng.standard_normal((L,), dtype=np.float32) * 0.02
    print(kernel(x, W, b).shape)


# revision 10
# speedup vs baseline: 1.4787x; 1.0054x over previous
"""Banded local-linear layer (nn_LocalLinearLayer) on 8 trn2 NeuronCores.

out[b, o, c] = sum_p W[o, p] * xpad[b, c, p] + bias[o],  band p in [o, o+25)
xpad = edge-replicate pad of x along L (first/last 12 rows duplicated).

Strategy (v6):
  - Tensor-parallel over output rows: core c owns out rows [512c, 512c+512)
    for ALL batches/channels -> banded weight is sharded 8-way (~136 KB/core)
    instead of replicated (1.06 MB/core).
  - 5 output tiles per core of M=104 rows (last 96): tile t contracts xpad
    rows [104t, 104t+128) -> ONE K=128 banded weight per tile, streamed as 4
    PSUM-bank matmuls of 512 columns.  No tiny corner matmuls (they stream
    full columns for 32 rows and read as idle to the PE clock governor).
  - bf16 operands (1 col/cycle on the PE), fp32 PSUM, fp16 output.
  - x tiles DMA'd individually on the Sync HWDGE ring so tile t's matmuls
    start as soon as its rows land; weights/bias ride the Scalar HWDGE ring;
    per-tile output DMAs ride the GpSimd SWDGE ring (measured 362 GB/s).
  - PSUM->SBUF bias-add copies split ScalarE/VectorE half-and-half into
    SEPARATE lo/hi SBUF tiles (a shared tile created a false cross-engine
    dependency that serialized the copies); lo-half stores ride GpSimd
    (SWDGE), hi-half stores ride the Sync HWDGE ring.
  - 5 dummy matmuls on a zeroed scratch tile lift the PE HAM clock gate
    (1.2 -> 2.4 GHz) during the initial DMA wait.
"""

import sys

for _p in ("/opt/trn_rl_repo",):
    if _p not in sys.path:
        sys.path.insert(0, _p)

import ml_dtypes
import numpy as np

import concourse.bass as bass
import concourse.tile as tile
from concourse import bacc, mybir
from concourse.bass_utils import run_bass_kernel_spmd

L = 4096
WIN = 25
PAD = (WIN - 1) // 2  # 12
PADDED = L + 2 * PAD  # 4120
B = 32
C = 64
NCORES = 8
P = 128
ROWS_PC = L // NCORES  # 512 output rows per core
M = P - (WIN - 1)  # 104 output rows per tile
NT = (ROWS_PC + M - 1) // M  # 5 tiles per core
M_LAST = ROWS_PC - (NT - 1) * M  # 96
NFREE = B * C  # 2048
NB = NFREE // 512  # 4 psum banks per tile

F32 = mybir.dt.float32
F16 = mybir.dt.float16
BF16 = mybir.dt.bfloat16
NPBF16 = np.dtype(ml_dtypes.bfloat16)


def _host_weights(W: np.ndarray, b: np.ndarray):
    """Band-extract and shard W/b by output row.

    wk[j, o] = W[o, o+j] is the dense band (j in [0, WIN)).
    Per core: w1[k, t, m] = wk[k-m, o0+m] for 0 <= k-m < WIN (o0 = 512c+104t),
    bias[m, t] = b[o0+m].  m beyond the tile's rows stays zero.
    """
    o = np.arange(L)
    wk = W[o[:, None], o[:, None] + np.arange(WIN)[None, :]].T  # [WIN, L]
    w1s, biass = [], []
    for c in range(NCORES):
        w1 = np.zeros((P, NT, M), np.float32)
        bias = np.zeros((M, NT), np.float32)
        for t in range(NT):
            o0 = c * ROWS_PC + t * M
            mt = min(M, ROWS_PC - t * M)
            for j in range(WIN):
                m = np.arange(0, mt)
                w1[m + j, t, m] = wk[j, o0 + m]
            bias[:mt, t] = b[o0 : o0 + mt]
        w1s.append(w1.astype(NPBF16))
        biass.append(bias)
    return w1s, biass


def _host_x(x: np.ndarray):
    """x [B, L, C] f32 -> per-core [128, NT, B*C] bf16 overlapping xpad tiles."""
    xp = np.concatenate([x[:, :PAD], x, x[:, -PAD:]], axis=1)  # [B, PADDED, C]
    xpad = np.zeros((B, NCORES * ROWS_PC + P, C), np.float32)
    xpad[:, :PADDED] = xp
    xh = []
    for c in range(NCORES):
        t = np.empty((P, NT, B, C), np.float32)
        for ti in range(NT):
            r0 = c * ROWS_PC + ti * M
            t[:, ti] = xpad[:, r0 : r0 + P].transpose(1, 0, 2)
        xh.append(
            np.ascontiguousarray(t.reshape(P, NT, NFREE).astype(NPBF16))
        )
    return xh


def _build_nc():
    nc = bacc.Bacc("TRN2", target_bir_lowering=False, debug=False, num_devices=NCORES)
    x_d = nc.dram_tensor("x", [P, NT, NFREE], BF16, kind="ExternalInput").ap()
    w1_d = nc.dram_tensor("w1", [P, NT, M], BF16, kind="ExternalInput").ap()
    bias_d = nc.dram_tensor("bias", [M, NT], F32, kind="ExternalInput").ap()
    half = NFREE // 2
    out_lo_d = nc.dram_tensor("out_lo", [M, NT, half], F16, kind="ExternalOutput").ap()
    out_hi_d = nc.dram_tensor("out_hi", [M, NT, half], F16, kind="ExternalOutput").ap()

    with tile.TileContext(nc) as tc:
        with (
            tc.tile_pool(name="main", bufs=1) as pool,
            tc.tile_pool(name="ps", bufs=2, space=bass.MemorySpace.PSUM) as pspool,
        ):
            x_s = pool.tile([P, NT, NFREE], BF16)
            w1_s = pool.tile([P, NT, M], BF16)
            bias_s = pool.tile([M, NT], F32)
            out_lo = pool.tile([M, NT, half], F16)
            out_hi = pool.tile([M, NT, half], F16)
            scratch = pool.tile([P, 512], BF16)

            # per-tile x loads on the Sync HWDGE ring; small loads on Scalar's
            for t in range(NT):
                nc.sync.dma_start(x_s[:, t : t + 1], x_d[:, t : t + 1])
            nc.scalar.dma_start(w1_s[:], w1_d)
            nc.scalar.dma_start(bias_s[:], bias_d)

            # PE clock warm-up: dummy matmuls on zeroed scratch, no DMA deps
            nc.vector.memset(scratch[:], 0.0)
            ps_warm = pspool.tile([P, NFREE], F32, tag="psacc")
            for _ in range(5):
                nc.tensor.matmul(
                    ps_warm[:, 0:512], scratch[:, 0:P], scratch[:], start=True, stop=True
                )

            for t in range(NT):
                ps = pspool.tile([P, NFREE], F32, tag="psacc")
                for F in range(NB):
                    s = slice(F * 512, (F + 1) * 512)
                    nc.tensor.matmul(
                        ps[0:M, s], w1_s[:, t], x_s[:, t, s], start=True, stop=True
                    )
                nc.scalar.activation(
                    out_lo[:, t],
                    ps[0:M, 0:half],
                    mybir.ActivationFunctionType.Identity,
                    bias=bias_s[:, t : t + 1],
                )
                nc.vector.tensor_scalar_add(
                    out_hi[:, t], ps[0:M, half:NFREE], bias_s[:, t : t + 1]
                )
                nc.gpsimd.dma_start(out_lo_d[:, t], out_lo[:, t])
                nc.sync.dma_start(out_hi_d[:, t], out_hi[:, t])

    nc.compile()
    return nc


_NC = None


def _get_nc():
    global _NC
    if _NC is None:
        _NC = _build_nc()
    return _NC


def _make_in_maps(x, W, b):
    w1s, biass = _host_weights(
        np.asarray(W, dtype=np.float32), np.asarray(b, dtype=np.float32)
    )
    xh = _host_x(np.asarray(x, dtype=np.float32))
    return [
        {"x": xh[c], "w1": w1s[c], "bias": biass[c]} for c in range(NCORES)
    ]


def _gather(results):
    out = np.empty((B, L, C), np.float32)
    half = NFREE // 2
    hb = B // 2  # batches per half
    for c, r in enumerate(results):
        oh = np.empty((M, NT, B, C), np.float32)
        oh[:, :, :hb] = np.asarray(r["out_lo"]).reshape(M, NT, hb, C)
        oh[:, :, hb:] = np.asarray(r["out_hi"]).reshape(M, NT, hb, C)
        for t in range(NT):
            mt = min(M, ROWS_PC - t * M)
            r0 = c * ROWS_PC + t * M
            out[:, r0 : r0 + mt] = oh[:mt, t].transpose(1, 0, 2)
    return out


def kernel(x: np.ndarray, W: np.ndarray, b: np.ndarray) -> np.ndarray:
    nc = _get_nc()
    res = run_bass_kernel_spmd(nc, _make_in_maps(x, W, b), list(range(NCORES)))
    return _gather(res.results)


if __name__ == "__main__":
    rng = np.random.default_rng(0)
    x = rng.standard_normal((B, L, C), dtype=np.float32)
    W = rng.standard_normal((L, PADDED), dtype=np.float32) * 0.02
    b = rng.standard_normal((L,), dtype=np.float32) * 0.02
    print(kernel(x, W, b).shape)
